# revision 21
# baseline (speedup 1.0000x reference)
"""Invariant particle attention v2 — descriptor-light, bf16, fewer DVE ops.

Key changes vs baseline:
  * Host repacks q/k/v into one bf16 array x[N, 3, H, C'] (x-order q,v,k)
    with per-head channel order c' = [8 scalars, then f-major vectors
    8+30f+v]. Per-particle rows are 6KB contiguous -> 128 descriptors per
    tile DMA instead of 1024 512B ones.
  * Lorentz transform on DVE as 16 merged scalar_tensor_tensor ops per
    tile (3D free AP over (x, h, v), contiguous 30-elem runs, bf16 2x)
    using the identity linv = diag(s) @ inv: transform k with inv too,
    then flip the sign of k's spatial components.
  * q_g/k_g transposed per head on PE (bf16), S^T = kgT^T qgT in bf16,
    exp on ACT -> P bf16, PV with ones column for the denominator.
  * Output written as [N, H, C'] f32 (4KB rows); host transposes back to
    [H, N, C] and undoes the channel permutation.
"""

import contextlib
import sys

if "/opt/trn_rl_repo" not in sys.path:
    sys.path.insert(0, "/opt/trn_rl_repo")

import numpy as np
import ml_dtypes

import concourse.bass as bass
import concourse.mybir as mybir
import concourse.tile as tile
from concourse import bacc
from concourse.bass import ts
from concourse.bass_utils import run_bass_kernel_spmd
from concourse.masks import make_identity

F32 = mybir.dt.float32
BF16 = mybir.dt.bfloat16
MULT = mybir.AluOpType.mult
ADD = mybir.AluOpType.add

NS = 8    # scalar channels per head
NV = 30   # four-vector channels per head
P = 128


def build_nc(H=8, N=1024, C=128, repeat=1, merged_stt=True):
    NT = N // P
    CV = C - NS  # 120 vector channels
    scale = 1.0 / float(np.sqrt(C))

    nc = bacc.Bacc("TRN2", target_bir_lowering=False, debug=False)
    # x: [N, 3, H, C'] bf16, x-order (q, v, k), channels [s(8), 8+30f+v]
    x_d = nc.dram_tensor("x", [N, 3 * H * C], BF16, kind="ExternalInput")
    # ent: [N, 32] f32 = [inv(16 row-major), lam(16 row-major)]
    ent_d = nc.dram_tensor("ent", [N, 32], F32, kind="ExternalInput")
    # out: [N, H, C'] f32
    out_d = nc.dram_tensor("out", [N, H * C], BF16, kind="ExternalOutput")

    x_r = x_d.ap().rearrange("(t p) c -> t p c", p=P)
    ent_r = ent_d.ap().rearrange("(t p) e -> p t e", p=P)
    out_r = out_d.ap().rearrange("(t p) c -> t p c", p=P)

    with tile.TileContext(nc) as tc:
        with (
            tc.tile_pool(name="singles", bufs=1) as singles,
            tc.tile_pool(name="persist", bufs=1) as persist,
            tc.tile_pool(name="stage", bufs=2) as stage,
            tc.tile_pool(name="gbuf", bufs=2) as gbuf,
            tc.tile_pool(name="pbuf", bufs=2) as pbuf,
            tc.tile_pool(name="obuf", bufs=2) as obuf,
            tc.tile_pool(name="tps", bufs=2, space="PSUM") as tps_pool,
            tc.tile_pool(name="sps", bufs=2, space="PSUM") as sps_pool,
            tc.tile_pool(name="ups", bufs=2, space="PSUM") as ups_pool,
        ):
            idt = singles.tile([P, P], BF16)
            make_identity(nc, idt)

            loop_ctx = (
                tc.For_i(
                    0, repeat, 1,
                    hint_engines=(
                        mybir.EngineType.PE,
                        mybir.EngineType.DVE,
                        mybir.EngineType.Activation,
                        mybir.EngineType.SP,
                        mybir.EngineType.Pool,
                    ),
                )
                if repeat > 1
                else contextlib.nullcontext()
            )
            with loop_ctx:
                qgT = persist.tile([P, H, N], BF16, tag="qgT")  # [c, h, n]
                kgT = persist.tile([P, H, N], BF16, tag="kgT")
                vg = persist.tile([P, NT, H, 132], BF16, tag="vg")
                usb = persist.tile([P, NT, H, 132], BF16, tag="usb")
                rcp = persist.tile([P, NT, H], F32, tag="rcp")
                entT = persist.tile([P, NT, 32], F32, tag="entT")

                nc.sync.dma_start(out=entT, in_=ent_r)

                # ---------- phase 1: load + transform + transpose ----------
                for t in range(NT):
                    xt = stage.tile([P, 3, H, C], BF16, tag="xt")
                    nc.sync.dma_start(
                        out=xt.rearrange("p x h c -> p (x h c)"), in_=x_r[t]
                    )
                    g = gbuf.tile([P, 3, H, C], BF16, tag="g")

                    if merged_stt:
                        # o[x, h, mu-block] = sum_nu inv[mu,nu]*x[x, h, nu-block]
                        # as 4x-mode muls into tmp + 2x-mode adds (STT is 1x-only)
                        for mu in range(4):
                            tmp = gbuf.tile([P, 4, 3, H, NV], BF16, tag="tmp")
                            o = g[:, :, :, NS + NV * mu : NS + NV * (mu + 1)]
                            for nu in range(4):
                                col = entT[:, t, 4 * mu + nu : 4 * mu + nu + 1]
                                i0 = xt[:, :, :, NS + NV * nu : NS + NV * (nu + 1)]
                                nc.any.tensor_scalar_mul(tmp[:, nu], i0, col)
                            nc.any.tensor_add(o, tmp[:, 0], tmp[:, 1])
                            nc.any.tensor_add(tmp[:, 2], tmp[:, 2], tmp[:, 3])
                            nc.any.tensor_add(o, o, tmp[:, 2])
                        # k: linv = diag(1,-1,-1,-1) @ inv -> negate spatial rows
                        kneg = g[:, 2, :, NS + NV : C]
                        nc.any.tensor_scalar_mul(kneg, kneg, -1.0)
                    else:
                        for mu in range(4):
                            for x in range(3):
                                o = g[:, x, :, NS + NV * mu : NS + NV * (mu + 1)]
                                for nu in range(4):
                                    col = entT[:, t, 4 * mu + nu : 4 * mu + nu + 1]
                                    i0 = xt[:, x, :, NS + NV * nu : NS + NV * (nu + 1)]
                                    if nu == 0:
                                        nc.vector.tensor_scalar_mul(o, i0, col)
                                    else:
                                        nc.vector.scalar_tensor_tensor(
                                            out=o, in0=i0, scalar=col, in1=o,
                                            op0=MULT, op1=ADD,
                                        )
                        kneg = g[:, 2, :, NS + NV : C]
                        nc.vector.tensor_scalar_mul(kneg, kneg, -1.0)

                    # scalar channels pass through
                    nc.any.tensor_copy(g[:, :, :, 0:NS], xt[:, :, :, 0:NS])
                    # v block -> vg with ones column
                    nc.any.tensor_copy(vg[:, t, :, 0:C], g[:, 1])
                    nc.vector.memset(vg[:, t, :, C : C + 1], 1.0)

                    # transposes: q (x=0), k (x=2); 4 per PSUM tile, 1 copy each
                    for x, dst in ((0, qgT), (2, kgT)):
                        for hg in range(H // 4):
                            pg = tps_pool.tile([P, 4, P], BF16, tag="tp")
                            for hh in range(4):
                                nc.tensor.transpose(
                                    pg[:, hh], g[:, x, 4 * hg + hh, :], idt
                                )
                            nc.any.tensor_copy(
                                dst[:, 4 * hg : 4 * hg + 4, ts(t, P)], pg
                            )

                # ---------- phase 2: attention, S/PV software-pipelined ----
                def p2_scores(h):
                    pexp = pbuf.tile([P, NT, N], BF16, tag="pexp")  # [j, jt, i]
                    for jt in range(NT):
                        sT = sps_pool.tile([P, N], F32, tag="sT")
                        lhs = kgT[:, h, ts(jt, P)]
                        for half in range(N // 512):
                            nc.tensor.matmul(
                                sT[:, ts(half, 512)], lhs,
                                qgT[:, h, ts(half, 512)],
                                start=True, stop=True,
                            )
                        nc.scalar.activation(
                            pexp[:, jt, :], sT,
                            mybir.ActivationFunctionType.Exp, scale=scale,
                        )
                    return pexp

                def p2_pv(h, pexp):
                    for it in range(NT):
                        ups = ups_pool.tile([P, C + 1], F32, tag="ups")
                        for jt in range(NT):
                            nc.tensor.matmul(
                                ups, pexp[:, jt, ts(it, P)],
                                vg[:, jt, h, 0 : C + 1],
                                start=(jt == 0), stop=(jt == NT - 1),
                            )
                        # pinned to DVE: ACT is exp-bound during phase 2
                        nc.vector.tensor_copy(
                            usb[:, it, h, 0 : C + 1], ups[:, 0 : C + 1]
                        )

                # ---------- phase 3: output transform, one head-half ------
                # pinned to DVE so it hides under the ACT-bound exp phase
                def p3_half(hf):
                    h0 = hf * (H // 2)
                    HH = H // 2
                    nc.vector.reciprocal(
                        rcp[:, :, h0 : h0 + HH],
                        usb[:, :, h0 : h0 + HH, C],
                    )
                    for it in range(NT):
                        ob = obuf.tile([P, HH, C], BF16, tag="ob")
                        osb = obuf.tile([P, HH, C], BF16, tag="osb")
                        for mu in range(4):
                            tmp3 = obuf.tile([P, 4, HH, NV], BF16, tag="tmp3")
                            o = ob[:, :, NS + NV * mu : NS + NV * (mu + 1)]
                            for nu in range(4):
                                col = entT[:, it, 16 + 4 * mu + nu : 17 + 4 * mu + nu]
                                i0 = usb[:, it, h0 : h0 + HH, NS + NV * nu : NS + NV * (nu + 1)]
                                nc.vector.tensor_scalar_mul(tmp3[:, nu], i0, col)
                            nc.vector.tensor_add(tmp3[:, 0], tmp3[:, 0], tmp3[:, 1])
                            nc.vector.tensor_add(tmp3[:, 2], tmp3[:, 2], tmp3[:, 3])
                            nc.vector.tensor_add(o, tmp3[:, 0], tmp3[:, 2])
                        nc.vector.tensor_copy(
                            ob[:, :, 0:NS], usb[:, it, h0 : h0 + HH, 0:NS]
                        )
                        # normalize per head: osb = ob * (1/denom)
                        for hh in range(HH):
                            nc.vector.tensor_scalar_mul(
                                osb[:, hh, :], ob[:, hh, :],
                                rcp[:, it, h0 + hh : h0 + hh + 1],
                            )
                        nc.sync.dma_start(
                            out=out_r[it][:, ts(hf, HH * C)],
                            in_=osb.rearrange("p h c -> p (h c)"),
                        )

                # heads 0-3, then phase-3(half 0) while heads 4-7 run
                pexp_prev = p2_scores(0)
                for h in range(1, H // 2):
                    pexp_h = p2_scores(h)
                    p2_pv(h - 1, pexp_prev)
                    pexp_prev = pexp_h
                p2_pv(H // 2 - 1, pexp_prev)
                p3_half(0)
                pexp_prev = p2_scores(H // 2)
                for h in range(H // 2 + 1, H):
                    pexp_h = p2_scores(h)
                    p2_pv(h - 1, pexp_prev)
                    pexp_prev = pexp_h
                p2_pv(H - 1, pexp_prev)
                p3_half(1)

    nc.compile()
    return nc


# channel permutation: new c' -> old c  (per head)
def _perm_new2old():
    perm = np.empty(128, dtype=np.int64)
    perm[0:NS] = np.arange(NS)
    for f in range(4):
        for v in range(NV):
            perm[NS + NV * f + v] = NS + 4 * v + f
    return perm


_PERM_N2O = _perm_new2old()
_PERM_O2N = np.argsort(_PERM_N2O)  # old c -> new c'


def pack_inputs(q_local, k_local, v_local, lframes_matrices):
    """Returns per-core in_maps for run_bass_kernel_spmd."""
    B, H, N, C = q_local.shape
    s = np.array([1.0, -1.0, -1.0, -1.0], dtype=np.float32)
    L = np.asarray(lframes_matrices, dtype=np.float32)
    LT = np.swapaxes(L, -1, -2)
    invE = ((s[:, None] * s[None, :]) * LT).reshape(B, N, 16)
    lamE = L.reshape(B, N, 16)
    ent = np.concatenate([invE, lamE], axis=-1).astype(np.float32)  # [B, N, 32]

    # x[b, n, x, h, c'] from (q, v, k)[b, h, n, PERM[c']]
    stack = np.stack(
        [np.asarray(q_local), np.asarray(v_local), np.asarray(k_local)], axis=1
    )  # [B, 3, H, N, C] f32
    xp = stack[..., _PERM_N2O]                      # channel permute
    xp = np.transpose(xp, (0, 3, 1, 2, 4))          # [B, N, 3, H, C]
    xp = np.ascontiguousarray(xp, dtype=ml_dtypes.bfloat16)
    xp = xp.reshape(B, N, 3 * H * C)

    in_maps = []
    for b in range(B):
        in_maps.append({"x": xp[b], "ent": ent[b]})
    return in_maps


def unpack_output(res, B, H, N, C):
    """res.results[b]["out"] is [N, H*C'] f32 -> [B, H, N, C]."""
    outs = []
    for b in range(B):
        o = res.results[b]["out"].astype(np.float32).reshape(N, H, C)
        o = np.transpose(o, (1, 0, 2))[..., _PERM_O2N]
        outs.append(o)
    return np.ascontiguousarray(np.stack(outs, axis=0), dtype=np.float32)


_NC_CACHE = {}


def kernel(q_local, k_local, v_local, lframes_matrices, _results_hook=None):
    B, H, N, C = q_local.shape
    assert (B, H, N, C) == (8, 8, 1024, 128), (B, H, N, C)

    if "nc" not in _NC_CACHE:
        _NC_CACHE["nc"] = build_nc(H=H, N=N, C=C)
    nc = _NC_CACHE["nc"]

    in_maps = pack_inputs(q_local, k_local, v_local, lframes_matrices)
    res = run_bass_kernel_spmd(nc, in_maps, core_ids=list(range(B)))
    if _results_hook is not None:
        _results_hook(res)
    return unpack_output(res, B, H, N, C)

